# revision 30
# baseline (speedup 1.0000x reference)
"""DSA sparse attention (context-parallel variant) for Trainium2 via Bass/Tile.

Dense-rewrite algorithm (mathematically identical to the reference):
  w[s,t] = exp(sc[s,t])*ts[s,t] / sum_t' exp(sc)*ts   (softmax->*ts->renorm collapses)
  TS[s,j] = sum_t ts[s,t]*[idx[s,t]==j]  -> dense scatter of score values
  E[s,j]  = TS[s,j]*exp(scale*S[s,j]),  S = Q K^T (dense)
  O       = (E @ V) / rowsum(E)
Everything is computed in transposed layout (kv on partitions); O comes out
natural via E^T-stationary matmuls; rowsum(E) falls out of a ones-column
appended to V.

Layout/scheduling notes (the kernel is ACT-bound: exp over HPC*SQ*SKV =
8.4M elements/core at 1 elem/cycle/lane is the floor; everything else is
arranged to keep the other engines off the critical path):
  - host pre-TRANSPOSES q (pre-scaled) and k to [D, S] layout and appends
    a ones column to v: no on-chip transposes, all input DMAs contiguous.
  - host pre-builds the DENSE TS table (bf16, [128, NKV, SGRP] per
    s-group) and the kernel DMAs it instead of running 64 gpsimd
    local_scatters: the Pool engine (47us scatters + 11us drains per rep)
    drops to zero and carries only the output SWDGE DMAs.
  - S psum tiles are [128, 3, 512] (3 banks, double-buffered = 6) so each
    ACT exp call covers 1536 elements, amortizing the ~352-cycle ACT
    pipeline fill; the four EV accumulators are half-bank [128, 256] fp32
    tiles (2 banks total, consecutive matmuls alternate banks to avoid
    the PSUM accumulate RMW stall).  Total PSUM = exactly 8 banks.
  - phases run g-major: (h0,g0) (h1,g0) (h0,g1) (h1,g1); each phase's S^T
    matmuls are WOVEN with the EV matmuls of the PREVIOUS phase (all of
    whose exp+mul deps are long met, so the in-order PE queue never
    stalls); the EV pending at a phase's end carries into the next phase
    and across unroll segments.
  - the timing loop unrolls 4 reps per For_i iteration: the loop seam is
    an all-engine rendezvous plus a cold DMA restart (~25us) that the
    unroll amortizes; input DMAs are issued in WAR-clear order on the SP
    ring so later segments' transfers prefetch during earlier segments.
"""

import sys

sys.path.insert(0, "/opt/trn_rl_repo")

import numpy as np

import concourse.bass as bass
import concourse.bacc as bacc
import concourse.mybir as mybir
import concourse.tile as tile
from concourse.vector_clock import ScopedClock

# ---------------------------------------------------------------------------
# Patch: this walrus build encodes at most ONE sync-wait on a CTRL NO_STRUCT
# instruction; TileContext's tail drain carries one wait per live proc.  Split
# the waits across a chain of single-wait drains.
# ---------------------------------------------------------------------------


def _patched_drain_and_barrier(self, tick_clock, wait_clock):
    drain_inst = self.nc.sync.drain()
    wait_clock.add_sem_waits(
        drain_inst.ins, ScopedClock({None: tick_clock.global_clock})
    )
    si = drain_inst.ins.sync_info
    if si is not None and len(si.on_wait) > 1:
        waits = list(si.on_wait)
        drain_inst.ins.sync_info = mybir.SyncInfo(
            on_wait=waits[:1], on_update=list(si.on_update)
        )
        for i in range(1, len(waits)):
            extra = self.nc.sync.drain()
            extra.ins.sync_info = mybir.SyncInfo(on_wait=[waits[i]], on_update=[])
    self.nc.all_engine_barrier()
    assert self.sems is not None
    popped = self.nc._tile_sem_poison_stack.pop()
    assert popped is self._sem_poison
    self.nc.clear_and_free_semaphores(list(self.sems.allocated().values()))
    self.nc.all_engine_barrier()


tile.TileContext._drain_and_barrier = _patched_drain_and_barrier

FP = mybir.dt.float32
BF = mybir.dt.bfloat16


class Cfg:
    def __init__(self, HPC=2, SQ=1024, SKV=4096, D=128, TOPK=64):
        self.HPC = HPC  # heads per core
        self.SQ = SQ
        self.SKV = SKV
        self.D = D
        self.TOPK = TOPK
        self.NKV = SKV // 128  # kv chunks of 128
        self.NSB = SQ // 128  # query blocks of 128
        self.SHALF = 512  # s-group width (s-dim per group)
        self.scale = float(D) ** -0.5


# ---------------------------------------------------------------------------
# Program builder
# ---------------------------------------------------------------------------


def build_program(cfg, nmaxs=None, reps=1):
    nc = bacc.Bacc("TRN2", debug=False)
    HPC, SQ, SKV, D, NKV = cfg.HPC, cfg.SQ, cfg.SKV, cfg.D, cfg.NKV
    NGRP = SQ // cfg.SHALF

    qT = nc.dram_tensor("qT", [HPC, D, SQ], BF, kind="ExternalInput").ap()
    kT = nc.dram_tensor("kT", [HPC, D, SKV], BF, kind="ExternalInput").ap()
    # v arrives with the ones column pre-appended by the host: contiguous DMA
    va = nc.dram_tensor("va", [HPC, SKV, D + 1], BF, kind="ExternalInput").ap()
    ts = nc.dram_tensor(
        "ts", [NGRP, 128, NKV, cfg.SHALF], BF, kind="ExternalInput"
    ).ap()
    out = nc.dram_tensor("out", [HPC, SQ, D], FP, kind="ExternalOutput").ap()

    with tile.TileContext(nc) as tc:
        import contextlib

        ctx = contextlib.ExitStack()
        with ctx:
            tst_pool = ctx.enter_context(tc.tile_pool(name="tst", bufs=2))
            ktr_pool = ctx.enter_context(tc.tile_pool(name="ktr", bufs=2))
            et_pool = ctx.enter_context(tc.tile_pool(name="et", bufs=2))
            small_pool = ctx.enter_context(tc.tile_pool(name="small", bufs=4))
            out_pool = ctx.enter_context(tc.tile_pool(name="outp", bufs=4))
            s_psum = ctx.enter_context(tc.tile_pool(name="sps", bufs=2, space="PSUM"))
            ev_psum = ctx.enter_context(tc.tile_pool(name="evp", bufs=2, space="PSUM"))

            builder = _SegmentBuilder(
                nc, tc, cfg, qT, kT, va, ts, out,
                tst_pool, ktr_pool, et_pool, small_pool, out_pool,
                s_psum, ev_psum,
            )

            # The For_i seam is an all-engine rendezvous (~7us) plus a cold
            # DMA restart; unrolling several reps per iteration amortizes it.
            # Within the body the WAR-ordered input DMAs prefetch across
            # segments; the final segment's EV drains in-body, overlapping
            # the seam barrier where the PE would idle anyway.
            if reps == 1:
                L = builder.emit_loads()
                builder.emit_compute(L, last=True)
            else:
                UNROLL = next(u for u in (4, 2, 1) if reps % u == 0)
                with tc.For_i(
                    0, reps // UNROLL, 1,
                    hint_engines=(
                        mybir.EngineType.PE,
                        mybir.EngineType.DVE,
                        mybir.EngineType.Activation,
                        mybir.EngineType.SP,
                    ),
                ):
                    for _seg in range(UNROLL):
                        L = builder.emit_loads()
                        builder.emit_compute(L, last=_seg == UNROLL - 1)

    nc.compile()
    return nc


class _SegmentBuilder:
    def __init__(self, nc, tc, cfg, qT, kT, va, ts, out,
                 tst_pool, ktr_pool, et_pool, small_pool, out_pool,
                 s_psum, ev_psum):
        self.nc, self.cfg = nc, cfg
        self.qT, self.kT, self.va, self.ts, self.out = qT, kT, va, ts, out
        self.tst_pool, self.ktr_pool, self.et_pool = tst_pool, ktr_pool, et_pool
        self.small_pool, self.out_pool = small_pool, out_pool
        self.s_psum, self.ev_psum = s_psum, ev_psum
        self.prev = None  # EvState of the phase whose EV is pending
        self.TRIP = 3
        NKV = cfg.NKV
        self.groups = [3] * (NKV // 3) + ([NKV % 3] if NKV % 3 else [])
        self.nweave = -(-4 * NKV // len(self.groups))

    # ---------------- input DMAs (contiguous, host-prepped layouts) ---------
    # All inputs load through the in-order SP ring; the issue order below is
    # sorted by when each tile's previous-user readers finish (WAR clear
    # time), so the ring head never blocks a transfer that could have run.
    def emit_loads(self):
        nc, cfg = self.nc, self.cfg
        SQ, SKV, D, NKV, SGRP = cfg.SQ, cfg.SKV, cfg.D, cfg.NKV, cfg.SHALF

        def _load_qk(h):
            qtr = self.ktr_pool.tile([128, SQ], BF, tag="qtr", name="qtr")
            nc.sync.dma_start(qtr[:], self.qT[h])
            ktr = self.ktr_pool.tile([128, SKV], BF, tag="ktr", name="ktr")
            nc.sync.dma_start(ktr[:, 0 : SKV // 2], self.kT[h, :, 0 : SKV // 2])
            nc.sync.dma_start(ktr[:, SKV // 2 : SKV], self.kT[h, :, SKV // 2 :])
            return qtr, ktr

        def _load_v(h):
            vaug = self.ktr_pool.tile([128, NKV, D + 1], BF, tag="vaug",
                                      name="vaug")
            vview = self.va[h].rearrange("(n p) d -> p n d", p=128)
            for c in range(2):
                nc.sync.dma_start(
                    vaug[:, c * NKV // 2 : (c + 1) * NKV // 2, :],
                    vview[:, c * NKV // 2 : (c + 1) * NKV // 2, :],
                )
            return vaug

        def _load_ts(g):
            tst = self.tst_pool.tile([128, NKV, SGRP], BF, tag="tst",
                                     name="tst")
            step = NKV // 4
            for c in range(4):
                nc.sync.dma_start(
                    tst[:, c * step : (c + 1) * step, :],
                    self.ts[g, :, c * step : (c + 1) * step, :],
                )
            return tst

        # Leading slivers first: after the For_i seam the ring restarts cold,
        # so the first S triple needs only q + k[:, 0:512], and the first
        # multiply only ts chunks 0-3.  Slice the leading k/ts transfers
        # small and interleave them so compute restarts ~4us sooner.
        qtr0 = self.ktr_pool.tile([128, SQ], BF, tag="qtr", name="qtr")
        nc.sync.dma_start(qtr0[:], self.qT[0])
        ktr0 = self.ktr_pool.tile([128, SKV], BF, tag="ktr", name="ktr")
        nc.sync.dma_start(ktr0[:, 0:512], self.kT[0, :, 0:512])
        tst0 = self.tst_pool.tile([128, NKV, SGRP], BF, tag="tst", name="tst")
        nc.sync.dma_start(tst0[:, 0:4, :], self.ts[0, :, 0:4, :])
        nc.sync.dma_start(ktr0[:, 512:2048], self.kT[0, :, 512:2048])
        nc.sync.dma_start(tst0[:, 4:12, :], self.ts[0, :, 4:12, :])
        nc.sync.dma_start(ktr0[:, 2048:SKV], self.kT[0, :, 2048:SKV])
        qtr1, ktr1 = _load_qk(1)
        nc.sync.dma_start(tst0[:, 12:22, :], self.ts[0, :, 12:22, :])
        nc.sync.dma_start(tst0[:, 22:NKV, :], self.ts[0, :, 22:NKV, :])
        vaug0 = _load_v(0)
        tst1 = _load_ts(1)
        vaug1 = _load_v(1)
        return dict(qtrs=[qtr0, qtr1], ktrs=[ktr0, ktr1],
                    vaugs=[vaug0, vaug1], tsts=[tst0, tst1])

    def _ev_state(self, h, g, et, vaug):
        nc, cfg = self.nc, self.cfg
        D, NKV, NSBG = cfg.D, cfg.NKV, cfg.SHALF // 128
        builder = self

        class EvState:
            """EV accumulation for one phase.  Normally woven into the NEXT
            phase's S-stream (all exp+mul dependencies long met, so the
            in-order PE queue never stalls); the run's final phase instead
            self-weaves with a lag.  J-outer order: the four s-block
            accumulators rotate over four half-bank PSUM tiles laid out so
            consecutive matmuls alternate banks (avoids the accumulate RMW
            stall); each block is normalized and stored when its J-loop
            closes."""

            def __init__(self):
                self.pos = 0  # (J, b) steps emitted; J = pos//4, b = pos%4
                self.ops = [None] * NSBG

            def emit(self, n):
                self.emit_upto_pos(min(4 * NKV, self.pos + n))

            def emit_upto(self, j_ready):
                self.emit_upto_pos(4 * j_ready)

            def emit_upto_pos(self, pos_end):
                while self.pos < pos_end:
                    J, b = divmod(self.pos, 4)
                    if J == 0:
                        self.ops[b] = builder.ev_psum.tile(
                            [128, 256], FP, tag="evacc", name="evacc"
                        )
                    nc.tensor.matmul(
                        self.ops[b][:, 0 : D + 1],
                        et[:, J, b * 128 : (b + 1) * 128],
                        vaug[:, J, :],
                        start=(J == 0), stop=(J == NKV - 1),
                    )
                    if J == NKV - 1:
                        sb = g * NSBG + b
                        recip = builder.small_pool.tile(
                            [128, 1], FP, tag="recip", name="recip"
                        )
                        nc.vector.reciprocal(recip[:], self.ops[b][:, D : D + 1])
                        ot = builder.out_pool.tile([128, D], FP, tag="ot",
                                                   name="ot")
                        nc.vector.tensor_scalar_mul(
                            ot[:], self.ops[b][:, 0:D], recip[:]
                        )
                        # outputs ride the SP ring too: they are spread
                        # across the segment, so only the final phase's few
                        # stores sit ahead of the next segment's loads, while
                        # keeping Pool entirely out of the loop (its SWDGE
                        # drain otherwise lengthens the For_i rendezvous)
                        nc.sync.dma_start(
                            builder.out[h, sb * 128 : (sb + 1) * 128, :], ot[:]
                        )
                    self.pos += 1

            def finish(self):
                self.emit_upto_pos(4 * NKV)

        return EvState()

    def emit_compute(self, L, last=False):
        nc, cfg = self.nc, self.cfg
        NKV, SGRP, TRIP = cfg.NKV, cfg.SHALF, self.TRIP
        NGRP = cfg.SQ // SGRP
        phases = [(h, g) for g in range(NGRP) for h in range(cfg.HPC)]

        for pi, (h, g) in enumerate(phases):
            qtr, ktr, tst = L["qtrs"][h], L["ktrs"][h], L["tsts"][g]
            et = self.et_pool.tile([128, NKV, SGRP], BF, tag="et", name="et")
            sl = slice(g * SGRP, (g + 1) * SGRP)
            J = 0
            for gi, w in enumerate(self.groups):
                sp = self.s_psum.tile([128, TRIP, SGRP], FP, tag="sps",
                                      name="sps")
                for t in range(w):
                    nc.tensor.matmul(
                        sp[:, t, :],
                        ktr[:, (J + t) * 128 : (J + t + 1) * 128],
                        qtr[:, sl],
                        start=True, stop=True,
                    )
                if self.prev is not None:
                    self.prev.emit(self.nweave)
                nc.scalar.activation(
                    et[:, J : J + w, :], sp[:, 0:w, :],
                    mybir.ActivationFunctionType.Exp,
                )
                nc.vector.tensor_mul(
                    et[:, J : J + w, :], et[:, J : J + w, :], tst[:, J : J + w, :]
                )
                J += w
            if self.prev is not None:
                self.prev.finish()
            self.prev = self._ev_state(h, g, et, L["vaugs"][h])

        if last:
            # drain the final phase's EV unwoven; it overlaps the For_i
            # seam barrier where the PE would idle anyway
            self.prev.finish()
            self.prev = None


# ---------------------------------------------------------------------------
# Entry point: full unsharded inputs -> full output.
# Sharding: head-parallel, 2 heads per NeuronCore across 8 cores; the
# topk index/score tensors are shared by all cores.
# ---------------------------------------------------------------------------

_CACHE = {}


def make_in_maps(q, k, v, topk_indices, topk_scores, cfg):
    """Host-side prep: bf16 conversion, q pre-scaling + transpose, k
    transpose, dense TS table build.  Returns (in_maps, nmaxs)."""
    import ml_dtypes

    bf16 = ml_dtypes.bfloat16
    SQ, SKV, NKV, SGRP = cfg.SQ, cfg.SKV, cfg.NKV, cfg.SHALF
    NGRP = SQ // SGRP

    # dense TS[j, s] = sum of topk_scores over duplicate (s, j) selections
    idx = np.asarray(topk_indices)[0].astype(np.int64)          # [SQ, TOPK]
    sc = np.asarray(topk_scores, dtype=np.float32)[0]           # [SQ, TOPK]
    tsd = np.zeros((SKV, SQ), dtype=np.float32)                 # [j, s]
    s_arr = np.repeat(np.arange(SQ, dtype=np.int64), cfg.TOPK)
    np.add.at(tsd, (idx.reshape(-1), s_arr), sc.reshape(-1))
    # per group: [128, NKV, SGRP] with ts[p, J, s] = tsd[J*128 + p, g*SGRP + s]
    tsd = tsd.reshape(NKV, 128, NGRP, SGRP).transpose(2, 1, 0, 3)  # [g,p,J,s]
    ts_bf = np.ascontiguousarray(tsd.astype(bf16))

    qs = (np.asarray(q, dtype=np.float32) * (float(cfg.D) ** -0.5)).astype(bf16)
    kb = np.asarray(k, dtype=np.float32).astype(bf16)
    qsT = np.ascontiguousarray(qs[0].transpose(0, 2, 1))  # [H, D, SQ]
    kbT = np.ascontiguousarray(kb[0].transpose(0, 2, 1))  # [H, D, SKV]
    # v with a ones column appended (feeds the rowsum via the EV matmul)
    H = qs.shape[1]
    va = np.ones((H, SKV, cfg.D + 1), dtype=bf16)
    va[:, :, 0 : cfg.D] = np.asarray(v, dtype=np.float32)[0].astype(bf16)

    in_maps = []
    for i in range(8):
        m = {
            "qT": np.ascontiguousarray(qsT[2 * i : 2 * i + 2]),
            "kT": np.ascontiguousarray(kbT[2 * i : 2 * i + 2]),
            "va": np.ascontiguousarray(va[2 * i : 2 * i + 2]),
            "ts": ts_bf,
        }
        in_maps.append(m)
    return in_maps, ()


def kernel(q, k, v, topk_indices, topk_scores):
    q = np.asarray(q, dtype=np.float32)
    B, H, SQ, D = q.shape
    SKV = np.asarray(k).shape[2]
    TOPK = np.asarray(topk_indices).shape[-1]
    assert B == 1 and H == 16 and SQ == 1024 and SKV == 4096 and D == 128

    cfg = Cfg(HPC=H // 8, SQ=SQ, SKV=SKV, D=D, TOPK=TOPK)
    in_maps, nmaxs = make_in_maps(q, k, v, topk_indices, topk_scores, cfg)

    nc = _CACHE.get("v3")
    if nc is None:
        nc = build_program(cfg, list(nmaxs), reps=1)
        _CACHE["v3"] = nc

    from concourse.bass_utils import run_bass_kernel_spmd

    res = run_bass_kernel_spmd(nc, in_maps, list(range(8)))
    out = np.stack([res.results[i]["out"] for i in range(8)])
    return out.reshape(1, H, SQ, D).astype(np.float32)


# revision 31
# speedup vs baseline: 1.3195x; 1.3195x over previous
"""DSA sparse attention (context-parallel variant) for Trainium2 via Bass/Tile.

Dense-rewrite algorithm (mathematically identical to the reference):
  w[s,t] = exp(sc[s,t])*ts[s,t] / sum_t' exp(sc)*ts   (softmax->*ts->renorm collapses)
  TS[s,j] = sum_t ts[s,t]*[idx[s,t]==j]  -> dense scatter of score values
  E[s,j]  = TS[s,j]*exp(scale*S[s,j]),  S = Q K^T (dense)
  O       = (E @ V) / rowsum(E)
Everything is computed in transposed layout (kv on partitions); O comes out
natural via E^T-stationary matmuls; rowsum(E) falls out of a ones-column
appended to V.

Layout/scheduling notes (the kernel is ACT-bound: exp over HPC*SQ*SKV =
8.4M elements/core at 1 elem/cycle/lane is the floor; everything else is
arranged to keep the other engines off the critical path):
  - host pre-TRANSPOSES q (pre-scaled) and k to [D, S] layout and appends
    a ones column to v: no on-chip transposes, all input DMAs contiguous.
  - host pre-builds the DENSE TS table (bf16, [128, NKV, SGRP] per
    s-group) and the kernel DMAs it instead of running 64 gpsimd
    local_scatters: the Pool engine (47us scatters + 11us drains per rep)
    drops to zero and carries only the output SWDGE DMAs.
  - S psum tiles are [128, 3, 512] (3 banks, double-buffered = 6) so each
    ACT exp call covers 1536 elements, amortizing the ~352-cycle ACT
    pipeline fill; the four EV accumulators are half-bank [128, 256] fp32
    tiles (2 banks total, consecutive matmuls alternate banks to avoid
    the PSUM accumulate RMW stall).  Total PSUM = exactly 8 banks.
  - phases run g-major: (h0,g0) (h1,g0) (h0,g1) (h1,g1); each phase's S^T
    matmuls are WOVEN with the EV matmuls of the PREVIOUS phase (all of
    whose exp+mul deps are long met, so the in-order PE queue never
    stalls); the EV pending at a phase's end carries into the next phase
    and across unroll segments.
  - the timing loop unrolls 4 reps per For_i iteration: the loop seam is
    an all-engine rendezvous plus a cold DMA restart (~25us) that the
    unroll amortizes; input DMAs are issued in WAR-clear order on the SP
    ring so later segments' transfers prefetch during earlier segments.
"""

import sys

sys.path.insert(0, "/opt/trn_rl_repo")

import numpy as np

import concourse.bass as bass
import concourse.bacc as bacc
import concourse.mybir as mybir
import concourse.tile as tile
from concourse.vector_clock import ScopedClock

# ---------------------------------------------------------------------------
# Patch: this walrus build encodes at most ONE sync-wait on a CTRL NO_STRUCT
# instruction; TileContext's tail drain carries one wait per live proc.  Split
# the waits across a chain of single-wait drains.
# ---------------------------------------------------------------------------


def _patched_drain_and_barrier(self, tick_clock, wait_clock):
    drain_inst = self.nc.sync.drain()
    wait_clock.add_sem_waits(
        drain_inst.ins, ScopedClock({None: tick_clock.global_clock})
    )
    si = drain_inst.ins.sync_info
    if si is not None and len(si.on_wait) > 1:
        waits = list(si.on_wait)
        drain_inst.ins.sync_info = mybir.SyncInfo(
            on_wait=waits[:1], on_update=list(si.on_update)
        )
        for i in range(1, len(waits)):
            extra = self.nc.sync.drain()
            extra.ins.sync_info = mybir.SyncInfo(on_wait=[waits[i]], on_update=[])
    self.nc.all_engine_barrier()
    assert self.sems is not None
    popped = self.nc._tile_sem_poison_stack.pop()
    assert popped is self._sem_poison
    self.nc.clear_and_free_semaphores(list(self.sems.allocated().values()))
    self.nc.all_engine_barrier()


tile.TileContext._drain_and_barrier = _patched_drain_and_barrier

FP = mybir.dt.float32
BF = mybir.dt.bfloat16


class Cfg:
    def __init__(self, HPC=2, SQ=1024, SKV=4096, D=128, TOPK=64):
        self.HPC = HPC  # heads per core
        self.SQ = SQ
        self.SKV = SKV
        self.D = D
        self.TOPK = TOPK
        self.NKV = SKV // 128  # kv chunks of 128
        self.NSB = SQ // 128  # query blocks of 128
        self.SHALF = 512  # s-group width (s-dim per group)
        self.scale = float(D) ** -0.5


# ---------------------------------------------------------------------------
# Program builder
# ---------------------------------------------------------------------------


def build_program(cfg, nmaxs=None, reps=1):
    nc = bacc.Bacc("TRN2", debug=False)
    HPC, SQ, SKV, D, NKV = cfg.HPC, cfg.SQ, cfg.SKV, cfg.D, cfg.NKV
    NGRP = SQ // cfg.SHALF

    qT = nc.dram_tensor("qT", [HPC, D, SQ], BF, kind="ExternalInput").ap()
    kT = nc.dram_tensor("kT", [HPC, D, SKV], BF, kind="ExternalInput").ap()
    # v arrives with the ones column pre-appended by the host: contiguous DMA
    va = nc.dram_tensor("va", [HPC, SKV, D + 1], BF, kind="ExternalInput").ap()
    ts = nc.dram_tensor(
        "ts", [NGRP, 128, NKV, cfg.SHALF], BF, kind="ExternalInput"
    ).ap()
    out = nc.dram_tensor("out", [HPC, SQ, D], FP, kind="ExternalOutput").ap()

    with tile.TileContext(nc) as tc:
        import contextlib

        ctx = contextlib.ExitStack()
        with ctx:
            tst_pool = ctx.enter_context(tc.tile_pool(name="tst", bufs=2))
            ktr_pool = ctx.enter_context(tc.tile_pool(name="ktr", bufs=2))
            et_pool = ctx.enter_context(tc.tile_pool(name="et", bufs=2))
            small_pool = ctx.enter_context(tc.tile_pool(name="small", bufs=4))
            out_pool = ctx.enter_context(tc.tile_pool(name="outp", bufs=4))
            s_psum = ctx.enter_context(tc.tile_pool(name="sps", bufs=2, space="PSUM"))
            ev_psum = ctx.enter_context(tc.tile_pool(name="evp", bufs=2, space="PSUM"))

            builder = _SegmentBuilder(
                nc, tc, cfg, qT, kT, va, ts, out,
                tst_pool, ktr_pool, et_pool, small_pool, out_pool,
                s_psum, ev_psum,
            )

            # The For_i seam is an all-engine rendezvous (~7us) plus a cold
            # DMA restart; unrolling several reps per iteration amortizes it.
            # Within the body the WAR-ordered input DMAs prefetch across
            # segments; the final segment's EV drains in-body, overlapping
            # the seam barrier where the PE would idle anyway.
            if reps == 1:
                L = builder.emit_loads()
                builder.emit_compute(L, last=True)
            else:
                UNROLL = next(u for u in (4, 2, 1) if reps % u == 0)
                with tc.For_i(
                    0, reps // UNROLL, 1,
                    hint_engines=(
                        mybir.EngineType.PE,
                        mybir.EngineType.DVE,
                        mybir.EngineType.Activation,
                        mybir.EngineType.Pool,
                        mybir.EngineType.SP,
                    ),
                ):
                    for _seg in range(UNROLL):
                        L = builder.emit_loads()
                        builder.emit_compute(L, last=_seg == UNROLL - 1)

    nc.compile()
    return nc


class _SegmentBuilder:
    def __init__(self, nc, tc, cfg, qT, kT, va, ts, out,
                 tst_pool, ktr_pool, et_pool, small_pool, out_pool,
                 s_psum, ev_psum):
        self.nc, self.cfg = nc, cfg
        self.qT, self.kT, self.va, self.ts, self.out = qT, kT, va, ts, out
        self.tst_pool, self.ktr_pool, self.et_pool = tst_pool, ktr_pool, et_pool
        self.small_pool, self.out_pool = small_pool, out_pool
        self.s_psum, self.ev_psum = s_psum, ev_psum
        self.prev = None  # EvState of the phase whose EV is pending
        self.TRIP = 3
        NKV = cfg.NKV
        self.groups = [3] * (NKV // 3) + ([NKV % 3] if NKV % 3 else [])
        self.nweave = -(-4 * NKV // len(self.groups))

    # ---------------- input DMAs (contiguous, host-prepped layouts) ---------
    # All inputs load through the in-order SP ring; the issue order below is
    # sorted by when each tile's previous-user readers finish (WAR clear
    # time), so the ring head never blocks a transfer that could have run.
    def emit_loads(self):
        nc, cfg = self.nc, self.cfg
        SQ, SKV, D, NKV, SGRP = cfg.SQ, cfg.SKV, cfg.D, cfg.NKV, cfg.SHALF

        def _load_qk(h):
            qtr = self.ktr_pool.tile([128, SQ], BF, tag="qtr", name="qtr")
            nc.sync.dma_start(qtr[:], self.qT[h])
            ktr = self.ktr_pool.tile([128, SKV], BF, tag="ktr", name="ktr")
            nc.sync.dma_start(ktr[:, 0 : SKV // 2], self.kT[h, :, 0 : SKV // 2])
            nc.sync.dma_start(ktr[:, SKV // 2 : SKV], self.kT[h, :, SKV // 2 :])
            return qtr, ktr

        def _load_v(h):
            vaug = self.ktr_pool.tile([128, NKV, D + 1], BF, tag="vaug",
                                      name="vaug")
            vview = self.va[h].rearrange("(n p) d -> p n d", p=128)
            for c in range(2):
                nc.sync.dma_start(
                    vaug[:, c * NKV // 2 : (c + 1) * NKV // 2, :],
                    vview[:, c * NKV // 2 : (c + 1) * NKV // 2, :],
                )
            return vaug

        def _load_ts(g):
            tst = self.tst_pool.tile([128, NKV, SGRP], BF, tag="tst",
                                     name="tst")
            step = NKV // 4
            for c in range(4):
                nc.sync.dma_start(
                    tst[:, c * step : (c + 1) * step, :],
                    self.ts[g, :, c * step : (c + 1) * step, :],
                )
            return tst

        # Leading slivers first: after the For_i seam the ring restarts cold,
        # so the first S triple needs only q + k[:, 0:512], and the first
        # multiply only ts chunks 0-3.  Slice the leading k/ts transfers
        # small and interleave them so compute restarts ~4us sooner.
        qtr0 = self.ktr_pool.tile([128, SQ], BF, tag="qtr", name="qtr")
        nc.sync.dma_start(qtr0[:], self.qT[0])
        ktr0 = self.ktr_pool.tile([128, SKV], BF, tag="ktr", name="ktr")
        nc.sync.dma_start(ktr0[:, 0:512], self.kT[0, :, 0:512])
        tst0 = self.tst_pool.tile([128, NKV, SGRP], BF, tag="tst", name="tst")
        nc.sync.dma_start(tst0[:, 0:4, :], self.ts[0, :, 0:4, :])
        nc.sync.dma_start(ktr0[:, 512:2048], self.kT[0, :, 512:2048])
        nc.sync.dma_start(tst0[:, 4:12, :], self.ts[0, :, 4:12, :])
        nc.sync.dma_start(ktr0[:, 2048:SKV], self.kT[0, :, 2048:SKV])
        qtr1, ktr1 = _load_qk(1)
        nc.sync.dma_start(tst0[:, 12:22, :], self.ts[0, :, 12:22, :])
        nc.sync.dma_start(tst0[:, 22:NKV, :], self.ts[0, :, 22:NKV, :])
        vaug0 = _load_v(0)
        tst1 = _load_ts(1)
        vaug1 = _load_v(1)
        return dict(qtrs=[qtr0, qtr1], ktrs=[ktr0, ktr1],
                    vaugs=[vaug0, vaug1], tsts=[tst0, tst1])

    def _ev_state(self, h, g, et, vaug):
        nc, cfg = self.nc, self.cfg
        D, NKV, NSBG = cfg.D, cfg.NKV, cfg.SHALF // 128
        builder = self

        class EvState:
            """EV accumulation for one phase.  Normally woven into the NEXT
            phase's S-stream (all exp+mul dependencies long met, so the
            in-order PE queue never stalls); the run's final phase instead
            self-weaves with a lag.  J-outer order: the four s-block
            accumulators rotate over four half-bank PSUM tiles laid out so
            consecutive matmuls alternate banks (avoids the accumulate RMW
            stall); each block is normalized and stored when its J-loop
            closes."""

            def __init__(self):
                self.pos = 0  # (J, b) steps emitted; J = pos//4, b = pos%4
                self.ops = [None] * NSBG

            def emit(self, n):
                self.emit_upto_pos(min(4 * NKV, self.pos + n))

            def emit_upto(self, j_ready):
                self.emit_upto_pos(4 * j_ready)

            def emit_upto_pos(self, pos_end):
                while self.pos < pos_end:
                    J, b = divmod(self.pos, 4)
                    if J == 0:
                        self.ops[b] = builder.ev_psum.tile(
                            [128, 256], FP, tag="evacc", name="evacc"
                        )
                    nc.tensor.matmul(
                        self.ops[b][:, 0 : D + 1],
                        et[:, J, b * 128 : (b + 1) * 128],
                        vaug[:, J, :],
                        start=(J == 0), stop=(J == NKV - 1),
                    )
                    if J == NKV - 1:
                        sb = g * NSBG + b
                        recip = builder.small_pool.tile(
                            [128, 1], FP, tag="recip", name="recip"
                        )
                        nc.vector.reciprocal(recip[:], self.ops[b][:, D : D + 1])
                        ot = builder.out_pool.tile([128, D], FP, tag="ot",
                                                   name="ot")
                        nc.vector.tensor_scalar_mul(
                            ot[:], self.ops[b][:, 0:D], recip[:]
                        )
                        # outputs go through the Pool SWDGE ring so they
                        # never block input DMAs queued on the SP ring
                        nc.gpsimd.dma_start(
                            builder.out[h, sb * 128 : (sb + 1) * 128, :], ot[:]
                        )
                    self.pos += 1

            def finish(self):
                self.emit_upto_pos(4 * NKV)

        return EvState()

    def emit_compute(self, L, last=False):
        nc, cfg = self.nc, self.cfg
        NKV, SGRP, TRIP = cfg.NKV, cfg.SHALF, self.TRIP
        NGRP = cfg.SQ // SGRP
        phases = [(h, g) for g in range(NGRP) for h in range(cfg.HPC)]

        for pi, (h, g) in enumerate(phases):
            qtr, ktr, tst = L["qtrs"][h], L["ktrs"][h], L["tsts"][g]
            et = self.et_pool.tile([128, NKV, SGRP], BF, tag="et", name="et")
            sl = slice(g * SGRP, (g + 1) * SGRP)
            J = 0
            for gi, w in enumerate(self.groups):
                sp = self.s_psum.tile([128, TRIP, SGRP], FP, tag="sps",
                                      name="sps")
                for t in range(w):
                    nc.tensor.matmul(
                        sp[:, t, :],
                        ktr[:, (J + t) * 128 : (J + t + 1) * 128],
                        qtr[:, sl],
                        start=True, stop=True,
                    )
                if self.prev is not None:
                    self.prev.emit(self.nweave)
                nc.scalar.activation(
                    et[:, J : J + w, :], sp[:, 0:w, :],
                    mybir.ActivationFunctionType.Exp,
                )
                nc.vector.tensor_mul(
                    et[:, J : J + w, :], et[:, J : J + w, :], tst[:, J : J + w, :]
                )
                J += w
            if self.prev is not None:
                self.prev.finish()
            self.prev = self._ev_state(h, g, et, L["vaugs"][h])

        if last:
            # drain the final phase's EV unwoven; it overlaps the For_i
            # seam barrier where the PE would idle anyway
            self.prev.finish()
            self.prev = None


# ---------------------------------------------------------------------------
# Entry point: full unsharded inputs -> full output.
# Sharding: head-parallel, 2 heads per NeuronCore across 8 cores; the
# topk index/score tensors are shared by all cores.
# ---------------------------------------------------------------------------

_CACHE = {}


def make_in_maps(q, k, v, topk_indices, topk_scores, cfg):
    """Host-side prep: bf16 conversion, q pre-scaling + transpose, k
    transpose, dense TS table build.  Returns (in_maps, nmaxs)."""
    import ml_dtypes

    bf16 = ml_dtypes.bfloat16
    SQ, SKV, NKV, SGRP = cfg.SQ, cfg.SKV, cfg.NKV, cfg.SHALF
    NGRP = SQ // SGRP

    # dense TS[j, s] = sum of topk_scores over duplicate (s, j) selections
    idx = np.asarray(topk_indices)[0].astype(np.int64)          # [SQ, TOPK]
    sc = np.asarray(topk_scores, dtype=np.float32)[0]           # [SQ, TOPK]
    tsd = np.zeros((SKV, SQ), dtype=np.float32)                 # [j, s]
    s_arr = np.repeat(np.arange(SQ, dtype=np.int64), cfg.TOPK)
    np.add.at(tsd, (idx.reshape(-1), s_arr), sc.reshape(-1))
    # per group: [128, NKV, SGRP] with ts[p, J, s] = tsd[J*128 + p, g*SGRP + s]
    tsd = tsd.reshape(NKV, 128, NGRP, SGRP).transpose(2, 1, 0, 3)  # [g,p,J,s]
    ts_bf = np.ascontiguousarray(tsd.astype(bf16))

    qs = (np.asarray(q, dtype=np.float32) * (float(cfg.D) ** -0.5)).astype(bf16)
    kb = np.asarray(k, dtype=np.float32).astype(bf16)
    qsT = np.ascontiguousarray(qs[0].transpose(0, 2, 1))  # [H, D, SQ]
    kbT = np.ascontiguousarray(kb[0].transpose(0, 2, 1))  # [H, D, SKV]
    # v with a ones column appended (feeds the rowsum via the EV matmul)
    H = qs.shape[1]
    va = np.ones((H, SKV, cfg.D + 1), dtype=bf16)
    va[:, :, 0 : cfg.D] = np.asarray(v, dtype=np.float32)[0].astype(bf16)

    in_maps = []
    for i in range(8):
        m = {
            "qT": np.ascontiguousarray(qsT[2 * i : 2 * i + 2]),
            "kT": np.ascontiguousarray(kbT[2 * i : 2 * i + 2]),
            "va": np.ascontiguousarray(va[2 * i : 2 * i + 2]),
            "ts": ts_bf,
        }
        in_maps.append(m)
    return in_maps, ()


def kernel(q, k, v, topk_indices, topk_scores):
    q = np.asarray(q, dtype=np.float32)
    B, H, SQ, D = q.shape
    SKV = np.asarray(k).shape[2]
    TOPK = np.asarray(topk_indices).shape[-1]
    assert B == 1 and H == 16 and SQ == 1024 and SKV == 4096 and D == 128

    cfg = Cfg(HPC=H // 8, SQ=SQ, SKV=SKV, D=D, TOPK=TOPK)
    in_maps, nmaxs = make_in_maps(q, k, v, topk_indices, topk_scores, cfg)

    nc = _CACHE.get("v3")
    if nc is None:
        nc = build_program(cfg, list(nmaxs), reps=1)
        _CACHE["v3"] = nc

    from concourse.bass_utils import run_bass_kernel_spmd

    res = run_bass_kernel_spmd(nc, in_maps, list(range(8)))
    out = np.stack([res.results[i]["out"] for i in range(8)])
    return out.reshape(1, H, SQ, D).astype(np.float32)
